# revision 25
# baseline (speedup 1.0000x reference)
"""Trainium2 Bass kernel for nn_AttentionConv (dense_transformer).

Sharding: data-parallel over batch — 8 NeuronCores, one batch image each.

Per-core dataflow (T=3136 tokens = 56x56, C=384, 6 heads x 64):
  - x shipped pre-transposed from host as xT [C, 58*58] bf16 (zero-padded).
  - Q depthwise 3x3 conv + BN hybrid: ctiles 0-1 off-PE (GPSIMD scales 5
    taps into tmp tiles via tensor_scalar, DVE accumulates: tensor_scalar +
    3 scalar_tensor_tensor + 5 tensor_tensor adds, bf16), ctile 2 on PE as
    diagonal-stationary matmuls. BN bias + cast on ACT. This fills the
    DVE/GPSIMD idle window while PE runs the K/V phase, and shrinks PE's
    conv share.
  - K/V stride-2 convs on PE: 9 shifted diagonal-stationary matmuls
    accumulate in PSUM (diagonals built on ACT from identity x per-channel
    scale), BN bias folded in at the ACT evacuation.
  - K projection -> kh^T [o, T2] (ACT evac). V projection emitted
    TRANSPOSED (stationary = vf t-tile, moving = wv) producing vh^T [t, o]
    directly into vhT with a ones column per head (softmax denominator
    trick); no PE transposes.
  - Q projection on PE (softmax scale folded into wq) -> qh^T [o, T],
    chunked; chunks 2-6 are interleaved into attention band 0's head slots.
  - Attention per head: scores^T [t, q] = kh^T.T @ qh^T on PE, exp on ACT
    (no max-subtraction: |scores| << 1 by construction), o^T [65, q] =
    [vh | ones]^T @ e^T accumulated over t tiles. Denominator (psum row
    64) -> reciprocal_approx_fast on DVE -> partition_broadcast on GPSIMD
    (no DRAM bounce, no DMA) -> per-head evac multiply on DVE.
  - Output projection in [l, o] orientation; evacuation adds b_last
    (replicated tile) on DVE and DMAs straight to DRAM rows. The previous
    band's tiles are interleaved into the next band's head slots.
"""
import sys

sys.path.insert(0, '/opt/trn_rl_repo')

import numpy as np

DIM = 384
HEADS = 6
D = 64
S = 56           # stride-1 spatial side
S2 = 28          # stride-2 spatial side
T = S * S        # 3136
T2 = S2 * S2     # 784
EPS = 1e-5
SCALE = DIM ** -0.5
NCORES = 8
CT = DIM // 128          # 3 channel tiles
NTT = (T2 + 127) // 128  # 7 kv t-tiles (last = 16 rows)
QB = 1024                # attention q band width
# the narrow tail band runs second so its serial denominator chain overlaps
# a dense band instead of dangling at the kernel tail
BANDS = [(0, 1024), (3072, 64), (1024, 1024), (2048, 1024)]
QCHUNKS = [(0, 512), (512, 512), (1024, 512), (1536, 512), (2048, 512),
           (2560, 512), (3072, 64)]

TAPS = [(dy, dx) for dy in (-1, 0, 1) for dx in (-1, 0, 1)]  # k=(dy+1)*3+(dx+1)


def build_program():
    import concourse.mybir as mybir
    from concourse import bacc
    from concourse.tile import TileContext
    dt = mybir.dt
    AF = mybir.ActivationFunctionType
    ALU = mybir.AluOpType

    nc = bacc.Bacc()

    SP = S + 2
    xT = nc.dram_tensor("xT", [DIM, SP * SP], dt.bfloat16,
                        kind="ExternalInput")
    qcp = nc.dram_tensor("qcp", [DIM, 10], dt.float32, kind="ExternalInput")
    wqt = nc.dram_tensor("wqt", [DIM, DIM], dt.bfloat16, kind="ExternalInput")
    wk2 = nc.dram_tensor("wk2", [DIM, HEADS, 128], dt.bfloat16,
                         kind="ExternalInput")
    wv = nc.dram_tensor("wv", [DIM, DIM], dt.bfloat16, kind="ExternalInput")
    kvb = nc.dram_tensor("kvb", [DIM, 2], dt.float32, kind="ExternalInput")
    wlt = nc.dram_tensor("wlt", [DIM, DIM], dt.bfloat16, kind="ExternalInput")
    blast = nc.dram_tensor("blast", [1, DIM], dt.float32, kind="ExternalInput")
    diag = nc.dram_tensor("diag", [128, 9 * CT * 3, 128], dt.bfloat16,
                          kind="ExternalInput")
    out = nc.dram_tensor("out", [T, DIM], dt.float32, kind="ExternalOutput")

    with TileContext(nc) as tc:
        with (
            tc.tile_pool(name="const", bufs=1) as cpool,
            tc.tile_pool(name="ework", bufs=3) as epool,
        ):
            # ---------------- Phase 0: loads ----------------
            xT_sb = cpool.tile([128, CT, SP, SP], dt.bfloat16)
            qcp_sb = cpool.tile([128, CT, 10], dt.float32)
            kvb_sb = cpool.tile([128, CT, 2], dt.float32)
            wqt_sb = cpool.tile([128, CT, DIM], dt.bfloat16)
            wk2_sb = cpool.tile([128, CT, HEADS, 128], dt.bfloat16)
            wv_sb = cpool.tile([128, CT, DIM], dt.bfloat16)
            wlt_sb = cpool.tile([128, CT, DIM], dt.bfloat16)
            btile = cpool.tile([128, DIM], dt.float32)
            dk_sb = cpool.tile([128, 9 * CT, 128], dt.bfloat16)
            dv_sb = cpool.tile([128, 9 * CT, 128], dt.bfloat16)
            dq_sb = cpool.tile([128, 9 * CT, 128], dt.bfloat16)
            kf_sb = cpool.tile([128, CT, T2], dt.bfloat16)
            vf_sb = cpool.tile([128, CT, T2], dt.bfloat16)

            def csl(c):
                return slice(c * 128, (c + 1) * 128)

            # two parallel HWDGE streams: SP loads the K-conv path (needed
            # first), ACT loads the rest.
            nc.sync.dma_start(dk_sb[:], diag[:, 0:9 * CT, :])
            for c in range(CT):
                nc.sync.dma_start(
                    xT_sb[:, c, :, :],
                    xT[csl(c), :].rearrange("p (h w) -> p h w", w=SP))
            nc.sync.dma_start(dq_sb[:], diag[:, 9 * CT:18 * CT, :])
            nc.sync.dma_start(dv_sb[:], diag[:, 18 * CT:, :])
            for c in range(CT):
                nc.scalar.dma_start(qcp_sb[:, c, :], qcp[csl(c), :])
                nc.scalar.dma_start(kvb_sb[:, c, :], kvb[csl(c), :])
                nc.scalar.dma_start(wk2_sb[:, c, :, :], wk2[csl(c), :, :])
                nc.scalar.dma_start(wv_sb[:, c, :], wv[csl(c), :])
                nc.scalar.dma_start(wqt_sb[:, c, :], wqt[csl(c), :])
                nc.scalar.dma_start(wlt_sb[:, c, :], wlt[csl(c), :])
            nc.scalar.dma_start(btile[:],
                                blast[0:1, :].to_broadcast([128, DIM]))

            # persistent activations
            q_feat = cpool.tile([128, CT, T], dt.bfloat16)
            qh_sb = cpool.tile([128, CT, T], dt.bfloat16)
            kh2_sb = cpool.tile([128, HEADS, NTT * 128], dt.bfloat16)
            vhT_sb = cpool.tile([128, NTT, HEADS * 65], dt.bfloat16)
            o_sb = cpool.tile([128, CT, T], dt.bfloat16)

            v4 = vhT_sb[:].rearrange("p n (h c) -> p n h c", c=65)
            nc.gpsimd.memset(vhT_sb[:], 1.0)
            # zero the pad rows of the 16-row tail t-tile so AV can contract
            # over the full 128 partitions there (half-width stationaries run
            # the PE at half clock); stale e rows hit these zeros.
            nc.gpsimd.memset(vhT_sb[:, NTT - 1, :], 0.0)
            nc.vector.memset(v4[0:T2 - 6 * 128, NTT - 1, 0:HEADS, 64:65], 1.0)
            # zero the kh2 tail pad so the last scores tile is full-width:
            # its extra rows become exp(0)=1, nullified by the zeroed vhT pad.
            nc.gpsimd.memset(kh2_sb[:, :, T2:], 0.0)

            psA_cm = tc.tile_pool(name="psA", bufs=2, space="PSUM")
            psA = psA_cm.__enter__()
            psB_cm = tc.tile_pool(name="psB", bufs=2, space="PSUM")
            psB = psB_cm.__enter__()

            # ------------- Phase 2: K/V stride-2 conv + projections ---------
            def kv_conv(d_sb, f_sb, bias_col):
                for c in range(CT):
                    x5 = xT_sb[:, c, :, :].rearrange(
                        "p (h sy) (w sx) -> p h sy w sx", sy=2, sx=2)
                    ps = psA.tile([128, QB], dt.float32, tag="psA")
                    for half, (ha, hb) in enumerate(((0, 14), (14, 28))):
                        for k in range(9):
                            dy, dx = TAPS[k]
                            hoff, sy = ((0, 0) if dy == -1 else
                                        (0, 1) if dy == 0 else (1, 0))
                            woff, sx = ((0, 0) if dx == -1 else
                                        (0, 1) if dx == 0 else (1, 0))
                            nc.tensor.matmul(
                                ps[:, half * 512:half * 512 + 14 * S2],
                                d_sb[:, k * CT + c, :],
                                x5[:, ha + hoff:hb + hoff, sy,
                                   woff:woff + S2, sx],
                                start=(k == 0), stop=(k == 8))
                    nc.scalar.activation(
                        f_sb[:, c, :],
                        ps[:].rearrange("p (two u) -> p two u", two=2)
                        [:, :, 0:14 * S2],
                        AF.Identity,
                        bias=kvb_sb[:, c, bias_col:bias_col + 1])

            kv_conv(dk_sb, kf_sb, 0)
            # K projection -> kh2 [128, head, t]: head h's 64 dims land on
            # its parity rows, the other 64 rows are ZERO (host-zeroed wk2
            # columns) so scores can contract over the full 128 partitions
            # (the PE runs at half clock with 64-row stationaries).
            for h in range(HEADS):
                ps = psA.tile([128, QB], dt.float32, tag="psA")
                for half, (ha, hb) in enumerate(((0, 14), (14, 28))):
                    for c in range(CT):
                        nc.tensor.matmul(
                            ps[:, half * 512:half * 512 + 14 * S2],
                            wk2_sb[:, c, h, :],
                            kf_sb[:, c, ha * S2:hb * S2],
                            start=(c == 0), stop=(c == CT - 1))
                nc.scalar.activation(
                    kh2_sb[:, h, 0:T2]
                    .rearrange("p (two u) -> p two u", two=2),
                    ps[:].rearrange("p (two u) -> p two u", two=2)
                    [:, :, 0:14 * S2],
                    AF.Copy)

            # ---- Phase 1: Q conv on PE (diagonal stationaries, paired) ----
            QROWS = 8  # 8*56 = 448 free
            r0s = list(range(0, S, QROWS))
            for c in range(CT):
                x3 = xT_sb[:, c, :, :]
                for g in range(0, len(r0s), 2):
                    pair = r0s[g:g + 2]
                    ps = psA.tile([128, QB], dt.float32, tag="psA")
                    for half, r0 in enumerate(pair):
                        rows = min(QROWS, S - r0)
                        for k in range(9):
                            dy, dx = TAPS[k]
                            nc.tensor.matmul(
                                ps[:, half * 512:half * 512 + rows * S],
                                dq_sb[:, k * CT + c, :],
                                x3[:, 1 + dy + r0:1 + dy + r0 + rows,
                                   1 + dx:1 + dx + S],
                                start=(k == 0), stop=(k == 8))
                    if len(pair) == 2:
                        nc.scalar.activation(
                            q_feat[:, c, pair[0] * S:pair[0] * S
                                   + 2 * QROWS * S]
                            .rearrange("p (two u) -> p two u", two=2),
                            ps[:].rearrange("p (two u) -> p two u", two=2)
                            [:, :, 0:QROWS * S],
                            AF.Identity, bias=qcp_sb[:, c, 9:10])
                    else:
                        nc.scalar.activation(
                            q_feat[:, c, pair[0] * S:T],
                            ps[:, 0:(S - pair[0]) * S], AF.Identity,
                            bias=qcp_sb[:, c, 9:10])

            kv_conv(dv_sb, vf_sb, 1)
            # V projection TRANSPOSED: vh^T [t, o] = vf-tile^T @ wv, written
            # straight into the vhT layout (65-wide per head, ones preserved).
            for tt in range(NTT):
                tsz = min(128, T2 - tt * 128)
                ps = psB.tile([128, QB], dt.float32, tag="psB")
                for c in range(CT):
                    nc.tensor.matmul(
                        ps[0:tsz, 0:DIM],
                        vf_sb[:, c, tt * 128:tt * 128 + tsz],
                        wv_sb[:, c, :],
                        start=(c == 0), stop=(c == CT - 1))
                nc.scalar.activation(
                    v4[0:tsz, tt, 0:HEADS, 0:64],
                    ps[0:tsz, 0:DIM].rearrange("p (h c) -> p h c", c=64),
                    AF.Copy)

            # ---------------- Phase 3: Q projection (qh^T [o, T]) -----------
            def qproj_unit(lc, ot):
                lpos, lw = QCHUNKS[lc]
                osl = slice(ot * 128, (ot + 1) * 128)
                ps = psA.tile([128, QB], dt.float32, tag="psA")
                for c in range(CT):
                    nc.tensor.matmul(
                        ps[:, 0:lw], wqt_sb[:, c, osl],
                        q_feat[:, c, lpos:lpos + lw],
                        start=(c == 0), stop=(c == CT - 1))
                nc.vector.tensor_copy(qh_sb[:, ot, lpos:lpos + lw],
                                      ps[:, 0:lw])

            # chunks 0-1 paired into one PSUM tile (one DVE evac for 1024)
            for ot in range(CT):
                osl = slice(ot * 128, (ot + 1) * 128)
                ps = psA.tile([128, QB], dt.float32, tag="psA")
                for half in (0, 1):
                    lpos, lw = QCHUNKS[half]
                    for c in range(CT):
                        nc.tensor.matmul(
                            ps[:, half * 512:half * 512 + lw],
                            wqt_sb[:, c, osl],
                            q_feat[:, c, lpos:lpos + lw],
                            start=(c == 0), stop=(c == CT - 1))
                nc.vector.tensor_copy(qh_sb[:, ot, 0:1024], ps[:])

            # ---------------- Phase 4: attention ----------------
            def oproj_tile(lpos, lsz):
                ps = psB.tile([128, QB], dt.float32, tag="psB")
                for c in range(CT):
                    nc.tensor.matmul(
                        ps[0:lsz, 0:DIM], o_sb[:, c, lpos:lpos + lsz],
                        wlt_sb[:, c, :],
                        start=(c == 0), stop=(c == CT - 1))
                ostage = epool.tile([128, DIM], dt.float32, tag="ostage",
                                    bufs=2)
                nc.vector.tensor_tensor(
                    out=ostage[0:lsz, :], in0=ps[0:lsz, 0:DIM],
                    in1=btile[0:lsz, :], op=ALU.add)
                nc.sync.dma_start(out[lpos:lpos + lsz, :], ostage[0:lsz, :])

            def band_ltiles(qs, W):
                return [(qs + i, min(128, qs + W - (qs + i)))
                        for i in range(0, W, 128)]

            def head_tloop(h, qs, W, ps_o, obase):
                """scores -> exp -> o accumulation for one head over all
                t-tiles, software-pipelined so PE never stalls on ACT."""
                ot = h // 2
                hsl = slice(64 * (h % 2), 64 * (h % 2) + 64)

                def scores(tt):
                    ps_s = psA.tile([128, QB], dt.float32, tag="psA")
                    for sub in range(0, W, 512):
                        sw = min(512, W - sub)
                        nc.tensor.matmul(
                            ps_s[:, sub:sub + sw],
                            kh2_sb[:, h, tt * 128:tt * 128 + 128],
                            qh_sb[:, ot, qs + sub:qs + sub + sw],
                            start=True, stop=True)
                    return ps_s

                ps_s = scores(0)
                for tt in range(NTT):
                    e = epool.tile([128, QB], dt.bfloat16, tag="e",
                                   bufs=4)
                    nc.scalar.activation(e[:, 0:W], ps_s[:, 0:W], AF.Exp)
                    if tt + 1 < NTT:
                        ps_s = scores(tt + 1)
                    for sub in range(0, W, 512):
                        sw = min(512, W - sub)
                        nc.tensor.matmul(
                            ps_o[0:65, obase + sub:obase + sub + sw],
                            vhT_sb[0:128, tt, h * 65:h * 65 + 65],
                            e[0:128, sub:sub + sw],
                            start=(tt == 0), stop=(tt == NTT - 1))

            def norm_chain(ps_o, WW):
                """den row 64 -> SBUF -> reciprocal_approx_fast (DVE; its
                bitwise seed misreads PSUM directly) -> broadcast to 64
                partitions (GPSIMD). No DMA, no DRAM bounce."""
                den_sb = epool.tile([1, QB], dt.float32, tag="den", bufs=2)
                r_row = epool.tile([1, QB], dt.float32, tag="r_row", bufs=2)
                r_rep = epool.tile([64, QB], dt.float32, tag="r_rep", bufs=2)
                nc.vector.tensor_copy(den_sb[0:1, 0:WW], ps_o[64:65, 0:WW])
                nc.vector.reciprocal_approx_fast(r_row[0:1, 0:WW],
                                                 den_sb[0:1, 0:WW])
                nc.gpsimd.partition_broadcast(r_rep[0:64, 0:WW],
                                              r_row[0:1, 0:WW])
                return r_rep

            def evac_head(h, qs, W, ps_o, obase, r_rep, rbase):
                ot = h // 2
                hsl = slice(64 * (h % 2), 64 * (h % 2) + 64)
                nc.vector.tensor_tensor(
                    out=o_sb[hsl, ot, qs:qs + W],
                    in0=ps_o[0:64, obase:obase + W],
                    in1=r_rep[0:64, rbase:rbase + W],
                    op=ALU.mult)

            # filler work interleaved into head slots: band 0 gets the
            # remaining Q projection chunks; later bands get the previous
            # band's output-projection tiles.
            prev_band = None
            first_band = True
            for qs, W in BANDS:
                if first_band:
                    fillers = [(lambda lc=lc, ot=ot: qproj_unit(lc, ot))
                               for lc in range(2, len(QCHUNKS))
                               for ot in range(CT)]
                else:
                    fillers = [(lambda lp=lp, ls=ls: oproj_tile(lp, ls))
                               for lp, ls in band_ltiles(*prev_band)]

                if W * HEADS <= 512:
                    # narrow tail band: all heads share one PSUM tile and a
                    # single denominator chain.
                    ps_o = psB.tile([128, QB], dt.float32, tag="psB")
                    for h in range(HEADS):
                        head_tloop(h, qs, W, ps_o, h * W)
                        if h < len(fillers):
                            fillers[h]()
                    r_rep = norm_chain(ps_o, W * HEADS)
                    for h in range(HEADS):
                        evac_head(h, qs, W, ps_o, h * W, r_rep, h * W)
                else:
                    for h in range(HEADS):
                        ps_o = psB.tile([128, QB], dt.float32, tag="psB")
                        head_tloop(h, qs, W, ps_o, 0)
                        r_rep = norm_chain(ps_o, W)
                        evac_head(h, qs, W, ps_o, 0, r_rep, 0)
                        if h < len(fillers):
                            fillers[h]()

                for f in fillers[HEADS:]:
                    f()
                prev_band = (qs, W)
                first_band = False

            for lt in band_ltiles(*prev_band):
                oproj_tile(*lt)

            psB_cm.__exit__(None, None, None)
            psA_cm.__exit__(None, None, None)

    nc.compile()
    return nc


_CACHE = {}


def _prep_weights(inputs):
    import ml_dtypes
    bf16 = ml_dtypes.bfloat16
    f32 = np.float32

    def bn_fold(prefix):
        a = (np.asarray(inputs[f'bn{prefix}_s'], f32)
             / np.sqrt(np.asarray(inputs[f'bn{prefix}_v'], f32) + EPS))
        b = (np.asarray(inputs[f'bn{prefix}_b'], f32)
             - np.asarray(inputs[f'bn{prefix}_m'], f32) * a)
        return a.astype(f32), b.astype(f32)

    aq, bq = bn_fold('q')
    ak, bk = bn_fold('k')
    av, bv = bn_fold('v')

    conv_q = np.asarray(inputs['conv_q'], f32)[:, 0].reshape(DIM, 9)
    conv_k = np.asarray(inputs['conv_k'], f32)[:, 0].reshape(DIM, 9)
    conv_v = np.asarray(inputs['conv_v'], f32)[:, 0].reshape(DIM, 9)
    wq = np.asarray(inputs['wq'], f32)
    wk = np.asarray(inputs['wk'], f32)
    wv = np.asarray(inputs['wv'], f32)
    wl = np.asarray(inputs['w_last'], f32)

    qcp = np.zeros((DIM, 10), f32)
    qcp[:, :9] = conv_q * aq[:, None]
    qcp[:, 9] = bq

    qs_ = conv_q * aq[:, None]
    ks_ = conv_k * ak[:, None]
    vs_ = conv_v * av[:, None]
    diag = np.zeros((128, 81, 128), f32)
    ar = np.arange(128)
    for c in range(CT):
        for k in range(9):
            diag[ar, k * CT + c, ar] = ks_[128 * c + ar, k]
            diag[ar, 9 * CT + k * CT + c, ar] = qs_[128 * c + ar, k]
            diag[ar, 18 * CT + k * CT + c, ar] = vs_[128 * c + ar, k]

    wqt = np.ascontiguousarray((wq * SCALE).T).astype(bf16)  # [c, o]
    wkT = wk.T.astype(np.float32)  # [c, o]
    wk2 = np.zeros((DIM, HEADS, 128), np.float32)
    for h in range(HEADS):
        p = h % 2
        wk2[:, h, 64 * p:64 * p + 64] = wkT[:, 64 * h:64 * h + 64]
    wk2 = wk2.astype(bf16)
    wvt = np.ascontiguousarray(wv.T).astype(bf16)
    kvb = np.stack([bk, bv], axis=1).astype(f32)        # [c, 2]
    wlt = np.ascontiguousarray(wl.T).astype(bf16)
    blast = np.asarray(inputs['b_last'], f32).reshape(1, DIM)
    return {'qcp': qcp, 'wqt': wqt, 'wk2': wk2, 'wv': wvt,
            'kvb': kvb, 'wlt': wlt, 'blast': blast,
            'diag': diag.astype(bf16)}


def _prep_x(xb):
    """[T, C] f32 -> zero-padded transposed [C, 58*58] bf16."""
    import ml_dtypes
    pad = np.zeros((DIM, S + 2, S + 2), np.float32)
    pad[:, 1:1 + S, 1:1 + S] = xb.T.reshape(DIM, S, S)
    return pad.reshape(DIM, (S + 2) * (S + 2)).astype(ml_dtypes.bfloat16)


def kernel(**inputs):
    from concourse.bass_utils import run_bass_kernel_spmd

    if 'nc' not in _CACHE:
        _CACHE['nc'] = build_program()
    nc = _CACHE['nc']

    wmap = _prep_weights(inputs)
    x = np.asarray(inputs['x'], np.float32)  # [8, T, C]
    B = x.shape[0]

    in_maps = [{'xT': _prep_x(x[b]), **wmap} for b in range(B)]

    res = run_bass_kernel_spmd(nc, in_maps, list(range(NCORES)))
    outs = np.stack([np.asarray(res.results[b]['out']) for b in range(B)],
                    axis=0)
    return outs.astype(np.float32)


# revision 26
# speedup vs baseline: 1.1270x; 1.1270x over previous
"""Trainium2 Bass kernel for nn_AttentionConv (dense_transformer).

Sharding: data-parallel over batch — 8 NeuronCores, one batch image each.

Per-core dataflow (T=3136 tokens = 56x56, C=384, 6 heads x 64):
  - x shipped pre-transposed from host as xT [C, 58*58] bf16 (zero-padded).
  - Q depthwise 3x3 conv + BN hybrid: ctiles 0-1 off-PE (GPSIMD scales 5
    taps into tmp tiles via tensor_scalar, DVE accumulates: tensor_scalar +
    3 scalar_tensor_tensor + 5 tensor_tensor adds, bf16), ctile 2 on PE as
    diagonal-stationary matmuls. BN bias + cast on ACT. This fills the
    DVE/GPSIMD idle window while PE runs the K/V phase, and shrinks PE's
    conv share.
  - K/V stride-2 convs on PE: 9 shifted diagonal-stationary matmuls
    accumulate in PSUM (diagonals built on ACT from identity x per-channel
    scale), BN bias folded in at the ACT evacuation.
  - K projection -> kh^T [o, T2] (ACT evac). V projection emitted
    TRANSPOSED (stationary = vf t-tile, moving = wv) producing vh^T [t, o]
    directly into vhT with a ones column per head (softmax denominator
    trick); no PE transposes.
  - Q projection on PE (softmax scale folded into wq) -> qh^T [o, T],
    chunked; chunks 2-6 are interleaved into attention band 0's head slots.
  - Attention per head: scores^T [t, q] = kh^T.T @ qh^T on PE, exp on ACT
    (no max-subtraction: |scores| << 1 by construction), o^T [65, q] =
    [vh | ones]^T @ e^T accumulated over t tiles. Denominator (psum row
    64) -> reciprocal_approx_fast on DVE -> partition_broadcast on GPSIMD
    (no DRAM bounce, no DMA) -> per-head evac multiply on DVE.
  - Output projection in [l, o] orientation; evacuation adds b_last
    (replicated tile) on DVE and DMAs straight to DRAM rows. The previous
    band's tiles are interleaved into the next band's head slots.
"""
import sys

sys.path.insert(0, '/opt/trn_rl_repo')

import numpy as np

DIM = 384
HEADS = 6
D = 64
S = 56           # stride-1 spatial side
S2 = 28          # stride-2 spatial side
T = S * S        # 3136
T2 = S2 * S2     # 784
EPS = 1e-5
SCALE = DIM ** -0.5
NCORES = 8
CT = DIM // 128          # 3 channel tiles
NTT = (T2 + 127) // 128  # 7 kv t-tiles (last = 16 rows)
QB = 1024                # attention q band width
# the narrow tail band runs second so its serial denominator chain overlaps
# a dense band instead of dangling at the kernel tail
BANDS = [(0, 1024), (3072, 64), (1024, 1024), (2048, 1024)]
QCHUNKS = [(0, 512), (512, 512), (1024, 512), (1536, 512), (2048, 512),
           (2560, 512), (3072, 64)]

TAPS = [(dy, dx) for dy in (-1, 0, 1) for dx in (-1, 0, 1)]  # k=(dy+1)*3+(dx+1)


def build_program():
    import concourse.mybir as mybir
    from concourse import bacc
    from concourse.tile import TileContext
    dt = mybir.dt
    AF = mybir.ActivationFunctionType
    ALU = mybir.AluOpType

    nc = bacc.Bacc()

    SP = S + 2
    xT = nc.dram_tensor("xT", [DIM, SP * SP], dt.bfloat16,
                        kind="ExternalInput")
    qcp = nc.dram_tensor("qcp", [DIM, 10], dt.float32, kind="ExternalInput")
    wqt = nc.dram_tensor("wqt", [DIM, DIM], dt.bfloat16, kind="ExternalInput")
    wk2 = nc.dram_tensor("wk2", [DIM, HEADS, 128], dt.bfloat16,
                         kind="ExternalInput")
    wv = nc.dram_tensor("wv", [DIM, DIM], dt.bfloat16, kind="ExternalInput")
    kvb = nc.dram_tensor("kvb", [DIM, 2], dt.float32, kind="ExternalInput")
    wlt = nc.dram_tensor("wlt", [DIM, DIM], dt.bfloat16, kind="ExternalInput")
    blast = nc.dram_tensor("blast", [1, DIM], dt.float32, kind="ExternalInput")
    diag = nc.dram_tensor("diag", [128, 9 * CT * 3, 128], dt.bfloat16,
                          kind="ExternalInput")
    out = nc.dram_tensor("out", [T, DIM], dt.float32, kind="ExternalOutput")

    with TileContext(nc) as tc:
        with (
            tc.tile_pool(name="const", bufs=1) as cpool,
            tc.tile_pool(name="ework", bufs=3) as epool,
        ):
            # ---------------- Phase 0: loads ----------------
            xT_sb = cpool.tile([128, CT, SP, SP], dt.bfloat16)
            qcp_sb = cpool.tile([128, CT, 10], dt.float32)
            kvb_sb = cpool.tile([128, CT, 2], dt.float32)
            wqt_sb = cpool.tile([128, CT, DIM], dt.bfloat16)
            wk2_sb = cpool.tile([128, CT, HEADS, 128], dt.bfloat16)
            wv_sb = cpool.tile([128, CT, DIM], dt.bfloat16)
            wlt_sb = cpool.tile([128, CT, DIM], dt.bfloat16)
            btile = cpool.tile([128, DIM], dt.float32)
            dk_sb = cpool.tile([128, 9 * CT, 128], dt.bfloat16)
            dv_sb = cpool.tile([128, 9 * CT, 128], dt.bfloat16)
            dq_sb = cpool.tile([128, 9 * CT, 128], dt.bfloat16)
            kf_sb = cpool.tile([128, CT, T2], dt.bfloat16)
            vf_sb = cpool.tile([128, CT, T2], dt.bfloat16)

            def csl(c):
                return slice(c * 128, (c + 1) * 128)

            # two parallel HWDGE streams: SP loads the K-conv path (needed
            # first), ACT loads the rest.
            nc.sync.dma_start(dk_sb[:], diag[:, 0:9 * CT, :])
            for c in range(CT):
                nc.sync.dma_start(
                    xT_sb[:, c, :, :],
                    xT[csl(c), :].rearrange("p (h w) -> p h w", w=SP))
            nc.sync.dma_start(dq_sb[:], diag[:, 9 * CT:18 * CT, :])
            nc.sync.dma_start(dv_sb[:], diag[:, 18 * CT:, :])
            for c in range(CT):
                nc.scalar.dma_start(qcp_sb[:, c, :], qcp[csl(c), :])
                nc.scalar.dma_start(kvb_sb[:, c, :], kvb[csl(c), :])
                nc.scalar.dma_start(wk2_sb[:, c, :, :], wk2[csl(c), :, :])
                nc.scalar.dma_start(wv_sb[:, c, :], wv[csl(c), :])
                nc.scalar.dma_start(wqt_sb[:, c, :], wqt[csl(c), :])
                nc.scalar.dma_start(wlt_sb[:, c, :], wlt[csl(c), :])
            nc.scalar.dma_start(btile[:],
                                blast[0:1, :].to_broadcast([128, DIM]))

            # persistent activations
            q_feat = cpool.tile([128, CT, T], dt.bfloat16)
            qh_sb = cpool.tile([128, CT, T], dt.bfloat16)
            kh2_sb = cpool.tile([128, HEADS, NTT * 128], dt.bfloat16)
            vhT_sb = cpool.tile([128, NTT, HEADS * 65], dt.bfloat16)
            o_sb = cpool.tile([128, CT, T], dt.bfloat16)

            v4 = vhT_sb[:].rearrange("p n (h c) -> p n h c", c=65)
            nc.gpsimd.memset(vhT_sb[:], 1.0)
            # zero the pad rows of the 16-row tail t-tile so AV can contract
            # over the full 128 partitions there (half-width stationaries run
            # the PE at half clock); stale e rows hit these zeros.
            nc.gpsimd.memset(vhT_sb[:, NTT - 1, :], 0.0)
            nc.vector.memset(v4[0:T2 - 6 * 128, NTT - 1, 0:HEADS, 64:65], 1.0)
            # zero the kh2 tail pad so the last scores tile is full-width:
            # its extra rows become exp(0)=1, nullified by the zeroed vhT pad.
            nc.gpsimd.memset(kh2_sb[:, :, T2:], 0.0)

            psA_cm = tc.tile_pool(name="psA", bufs=2, space="PSUM")
            psA = psA_cm.__enter__()
            psB_cm = tc.tile_pool(name="psB", bufs=2, space="PSUM")
            psB = psB_cm.__enter__()

            # ------------- Phase 2: K/V stride-2 conv + projections ---------
            def kv_conv(d_sb, f_sb, bias_col):
                for c in range(CT):
                    x5 = xT_sb[:, c, :, :].rearrange(
                        "p (h sy) (w sx) -> p h sy w sx", sy=2, sx=2)
                    ps = psA.tile([128, QB], dt.float32, tag="psA")
                    for half, (ha, hb) in enumerate(((0, 14), (14, 28))):
                        for k in range(9):
                            dy, dx = TAPS[k]
                            hoff, sy = ((0, 0) if dy == -1 else
                                        (0, 1) if dy == 0 else (1, 0))
                            woff, sx = ((0, 0) if dx == -1 else
                                        (0, 1) if dx == 0 else (1, 0))
                            nc.tensor.matmul(
                                ps[:, half * 512:half * 512 + 14 * S2],
                                d_sb[:, k * CT + c, :],
                                x5[:, ha + hoff:hb + hoff, sy,
                                   woff:woff + S2, sx],
                                start=(k == 0), stop=(k == 8))
                    nc.scalar.activation(
                        f_sb[:, c, :],
                        ps[:].rearrange("p (two u) -> p two u", two=2)
                        [:, :, 0:14 * S2],
                        AF.Identity,
                        bias=kvb_sb[:, c, bias_col:bias_col + 1])

            kv_conv(dk_sb, kf_sb, 0)
            # K projection -> kh2 [128, head, t]: head h's 64 dims land on
            # its parity rows, the other 64 rows are ZERO (host-zeroed wk2
            # columns) so scores can contract over the full 128 partitions
            # (the PE runs at half clock with 64-row stationaries).
            for h in range(HEADS):
                ps = psA.tile([128, QB], dt.float32, tag="psA")
                for half, (ha, hb) in enumerate(((0, 14), (14, 28))):
                    for c in range(CT):
                        nc.tensor.matmul(
                            ps[:, half * 512:half * 512 + 14 * S2],
                            wk2_sb[:, c, h, :],
                            kf_sb[:, c, ha * S2:hb * S2],
                            start=(c == 0), stop=(c == CT - 1))
                nc.scalar.activation(
                    kh2_sb[:, h, 0:T2]
                    .rearrange("p (two u) -> p two u", two=2),
                    ps[:].rearrange("p (two u) -> p two u", two=2)
                    [:, :, 0:14 * S2],
                    AF.Copy)

            # ---- Phase 1: Q conv on PE (diagonal stationaries, paired) ----
            QROWS = 8  # 8*56 = 448 free
            r0s = list(range(0, S, QROWS))
            for c in range(CT):
                x3 = xT_sb[:, c, :, :]
                for g in range(0, len(r0s), 2):
                    pair = r0s[g:g + 2]
                    ps = psA.tile([128, QB], dt.float32, tag="psA")
                    for half, r0 in enumerate(pair):
                        rows = min(QROWS, S - r0)
                        for k in range(9):
                            dy, dx = TAPS[k]
                            nc.tensor.matmul(
                                ps[:, half * 512:half * 512 + rows * S],
                                dq_sb[:, k * CT + c, :],
                                x3[:, 1 + dy + r0:1 + dy + r0 + rows,
                                   1 + dx:1 + dx + S],
                                start=(k == 0), stop=(k == 8))
                    if len(pair) == 2:
                        nc.scalar.activation(
                            q_feat[:, c, pair[0] * S:pair[0] * S
                                   + 2 * QROWS * S]
                            .rearrange("p (two u) -> p two u", two=2),
                            ps[:].rearrange("p (two u) -> p two u", two=2)
                            [:, :, 0:QROWS * S],
                            AF.Identity, bias=qcp_sb[:, c, 9:10])
                    else:
                        nc.scalar.activation(
                            q_feat[:, c, pair[0] * S:T],
                            ps[:, 0:(S - pair[0]) * S], AF.Identity,
                            bias=qcp_sb[:, c, 9:10])

            kv_conv(dv_sb, vf_sb, 1)
            # V projection TRANSPOSED: vh^T [t, o] = vf-tile^T @ wv, written
            # straight into the vhT layout (65-wide per head, ones preserved).
            for tt in range(NTT):
                tsz = min(128, T2 - tt * 128)
                ps = psB.tile([128, QB], dt.float32, tag="psB")
                for c in range(CT):
                    nc.tensor.matmul(
                        ps[0:tsz, 0:DIM],
                        vf_sb[:, c, tt * 128:tt * 128 + tsz],
                        wv_sb[:, c, :],
                        start=(c == 0), stop=(c == CT - 1))
                nc.scalar.activation(
                    v4[0:tsz, tt, 0:HEADS, 0:64],
                    ps[0:tsz, 0:DIM].rearrange("p (h c) -> p h c", c=64),
                    AF.Copy)

            # ---------------- Phase 3: Q projection (qh^T [o, T]) -----------
            def qproj_chunk(lc):
                lpos, lw = QCHUNKS[lc]
                for ot in range(CT):
                    osl = slice(ot * 128, (ot + 1) * 128)
                    ps = psA.tile([128, QB], dt.float32, tag="psA")
                    for c in range(CT):
                        nc.tensor.matmul(
                            ps[:, 0:lw], wqt_sb[:, c, osl],
                            q_feat[:, c, lpos:lpos + lw],
                            start=(c == 0), stop=(c == CT - 1))
                    nc.vector.tensor_copy(qh_sb[:, ot, lpos:lpos + lw],
                                          ps[:, 0:lw])

            # chunks 0-1 paired into one PSUM tile (one DVE evac for 1024)
            for ot in range(CT):
                osl = slice(ot * 128, (ot + 1) * 128)
                ps = psA.tile([128, QB], dt.float32, tag="psA")
                for half in (0, 1):
                    lpos, lw = QCHUNKS[half]
                    for c in range(CT):
                        nc.tensor.matmul(
                            ps[:, half * 512:half * 512 + lw],
                            wqt_sb[:, c, osl],
                            q_feat[:, c, lpos:lpos + lw],
                            start=(c == 0), stop=(c == CT - 1))
                nc.vector.tensor_copy(qh_sb[:, ot, 0:1024], ps[:])

            # ---------------- Phase 4: attention ----------------
            def oproj_tile(lpos, lsz):
                ps = psB.tile([128, QB], dt.float32, tag="psB")
                for c in range(CT):
                    nc.tensor.matmul(
                        ps[0:lsz, 0:DIM], o_sb[:, c, lpos:lpos + lsz],
                        wlt_sb[:, c, :],
                        start=(c == 0), stop=(c == CT - 1))
                ostage = epool.tile([128, DIM], dt.float32, tag="ostage",
                                    bufs=2)
                nc.vector.tensor_tensor(
                    out=ostage[0:lsz, :], in0=ps[0:lsz, 0:DIM],
                    in1=btile[0:lsz, :], op=ALU.add)
                nc.sync.dma_start(out[lpos:lpos + lsz, :], ostage[0:lsz, :])

            def band_ltiles(qs, W):
                return [(qs + i, min(128, qs + W - (qs + i)))
                        for i in range(0, W, 128)]

            def head_tloop(h, qs, W, ps_o, obase):
                """scores -> exp -> o accumulation for one head over all
                t-tiles, software-pipelined so PE never stalls on ACT."""
                ot = h // 2
                hsl = slice(64 * (h % 2), 64 * (h % 2) + 64)

                def scores(tt):
                    ps_s = psA.tile([128, QB], dt.float32, tag="psA")
                    for sub in range(0, W, 512):
                        sw = min(512, W - sub)
                        nc.tensor.matmul(
                            ps_s[:, sub:sub + sw],
                            kh2_sb[:, h, tt * 128:tt * 128 + 128],
                            qh_sb[:, ot, qs + sub:qs + sub + sw],
                            start=True, stop=True)
                    return ps_s

                ps_s = scores(0)
                for tt in range(NTT):
                    e = epool.tile([128, QB], dt.bfloat16, tag="e")
                    nc.scalar.activation(e[:, 0:W], ps_s[:, 0:W], AF.Exp)
                    if tt + 1 < NTT:
                        ps_s = scores(tt + 1)
                    for sub in range(0, W, 512):
                        sw = min(512, W - sub)
                        nc.tensor.matmul(
                            ps_o[0:65, obase + sub:obase + sub + sw],
                            vhT_sb[0:128, tt, h * 65:h * 65 + 65],
                            e[0:128, sub:sub + sw],
                            start=(tt == 0), stop=(tt == NTT - 1))

            def norm_chain(ps_o, WW):
                """den row 64 -> SBUF -> reciprocal_approx_fast (DVE; its
                bitwise seed misreads PSUM directly) -> broadcast to 64
                partitions (GPSIMD). No DMA, no DRAM bounce."""
                den_sb = epool.tile([1, QB], dt.float32, tag="den", bufs=2)
                r_row = epool.tile([1, QB], dt.float32, tag="r_row", bufs=2)
                r_rep = epool.tile([64, QB], dt.float32, tag="r_rep", bufs=2)
                nc.vector.tensor_copy(den_sb[0:1, 0:WW], ps_o[64:65, 0:WW])
                nc.vector.reciprocal_approx_fast(r_row[0:1, 0:WW],
                                                 den_sb[0:1, 0:WW])
                nc.gpsimd.partition_broadcast(r_rep[0:64, 0:WW],
                                              r_row[0:1, 0:WW])
                return r_rep

            def evac_head(h, qs, W, ps_o, obase, r_rep, rbase):
                ot = h // 2
                hsl = slice(64 * (h % 2), 64 * (h % 2) + 64)
                nc.vector.tensor_tensor(
                    out=o_sb[hsl, ot, qs:qs + W],
                    in0=ps_o[0:64, obase:obase + W],
                    in1=r_rep[0:64, rbase:rbase + W],
                    op=ALU.mult)

            # filler work interleaved into head slots: band 0 gets the
            # remaining Q projection chunks; later bands get the previous
            # band's output-projection tiles.
            prev_band = None
            first_band = True
            for qs, W in BANDS:
                if first_band:
                    fillers = [(lambda lc=lc: qproj_chunk(lc))
                               for lc in range(2, len(QCHUNKS))]
                else:
                    fillers = [(lambda lp=lp, ls=ls: oproj_tile(lp, ls))
                               for lp, ls in band_ltiles(*prev_band)]

                if W * HEADS <= 512:
                    # narrow tail band: all heads share one PSUM tile and a
                    # single denominator chain.
                    ps_o = psB.tile([128, QB], dt.float32, tag="psB")
                    for h in range(HEADS):
                        head_tloop(h, qs, W, ps_o, h * W)
                        if h < len(fillers):
                            fillers[h]()
                    r_rep = norm_chain(ps_o, W * HEADS)
                    for h in range(HEADS):
                        evac_head(h, qs, W, ps_o, h * W, r_rep, h * W)
                else:
                    for h in range(HEADS):
                        ps_o = psB.tile([128, QB], dt.float32, tag="psB")
                        head_tloop(h, qs, W, ps_o, 0)
                        r_rep = norm_chain(ps_o, W)
                        evac_head(h, qs, W, ps_o, 0, r_rep, 0)
                        if h < len(fillers):
                            fillers[h]()

                for f in fillers[HEADS:]:
                    f()
                prev_band = (qs, W)
                first_band = False

            for lt in band_ltiles(*prev_band):
                oproj_tile(*lt)

            psB_cm.__exit__(None, None, None)
            psA_cm.__exit__(None, None, None)

    nc.compile()
    return nc


_CACHE = {}


def _prep_weights(inputs):
    import ml_dtypes
    bf16 = ml_dtypes.bfloat16
    f32 = np.float32

    def bn_fold(prefix):
        a = (np.asarray(inputs[f'bn{prefix}_s'], f32)
             / np.sqrt(np.asarray(inputs[f'bn{prefix}_v'], f32) + EPS))
        b = (np.asarray(inputs[f'bn{prefix}_b'], f32)
             - np.asarray(inputs[f'bn{prefix}_m'], f32) * a)
        return a.astype(f32), b.astype(f32)

    aq, bq = bn_fold('q')
    ak, bk = bn_fold('k')
    av, bv = bn_fold('v')

    conv_q = np.asarray(inputs['conv_q'], f32)[:, 0].reshape(DIM, 9)
    conv_k = np.asarray(inputs['conv_k'], f32)[:, 0].reshape(DIM, 9)
    conv_v = np.asarray(inputs['conv_v'], f32)[:, 0].reshape(DIM, 9)
    wq = np.asarray(inputs['wq'], f32)
    wk = np.asarray(inputs['wk'], f32)
    wv = np.asarray(inputs['wv'], f32)
    wl = np.asarray(inputs['w_last'], f32)

    qcp = np.zeros((DIM, 10), f32)
    qcp[:, :9] = conv_q * aq[:, None]
    qcp[:, 9] = bq

    qs_ = conv_q * aq[:, None]
    ks_ = conv_k * ak[:, None]
    vs_ = conv_v * av[:, None]
    diag = np.zeros((128, 81, 128), f32)
    ar = np.arange(128)
    for c in range(CT):
        for k in range(9):
            diag[ar, k * CT + c, ar] = ks_[128 * c + ar, k]
            diag[ar, 9 * CT + k * CT + c, ar] = qs_[128 * c + ar, k]
            diag[ar, 18 * CT + k * CT + c, ar] = vs_[128 * c + ar, k]

    wqt = np.ascontiguousarray((wq * SCALE).T).astype(bf16)  # [c, o]
    wkT = wk.T.astype(np.float32)  # [c, o]
    wk2 = np.zeros((DIM, HEADS, 128), np.float32)
    for h in range(HEADS):
        p = h % 2
        wk2[:, h, 64 * p:64 * p + 64] = wkT[:, 64 * h:64 * h + 64]
    wk2 = wk2.astype(bf16)
    wvt = np.ascontiguousarray(wv.T).astype(bf16)
    kvb = np.stack([bk, bv], axis=1).astype(f32)        # [c, 2]
    wlt = np.ascontiguousarray(wl.T).astype(bf16)
    blast = np.asarray(inputs['b_last'], f32).reshape(1, DIM)
    return {'qcp': qcp, 'wqt': wqt, 'wk2': wk2, 'wv': wvt,
            'kvb': kvb, 'wlt': wlt, 'blast': blast,
            'diag': diag.astype(bf16)}


def _prep_x(xb):
    """[T, C] f32 -> zero-padded transposed [C, 58*58] bf16."""
    import ml_dtypes
    pad = np.zeros((DIM, S + 2, S + 2), np.float32)
    pad[:, 1:1 + S, 1:1 + S] = xb.T.reshape(DIM, S, S)
    return pad.reshape(DIM, (S + 2) * (S + 2)).astype(ml_dtypes.bfloat16)


def kernel(**inputs):
    from concourse.bass_utils import run_bass_kernel_spmd

    if 'nc' not in _CACHE:
        _CACHE['nc'] = build_program()
    nc = _CACHE['nc']

    wmap = _prep_weights(inputs)
    x = np.asarray(inputs['x'], np.float32)  # [8, T, C]
    B = x.shape[0]

    in_maps = [{'xT': _prep_x(x[b]), **wmap} for b in range(B)]

    res = run_bass_kernel_spmd(nc, in_maps, list(range(NCORES)))
    outs = np.stack([np.asarray(res.results[b]['out']) for b in range(B)],
                    axis=0)
    return outs.astype(np.float32)


# revision 27
# speedup vs baseline: 1.1619x; 1.0310x over previous
"""Trainium2 Bass kernel for nn_AttentionConv (dense_transformer).

Sharding: data-parallel over batch — 8 NeuronCores, one batch image each.

Per-core dataflow (T=3136 tokens = 56x56, C=384, 6 heads x 64):
  - x shipped pre-transposed from host as xT [C, 58*58] bf16 (zero-padded).
  - Q depthwise 3x3 conv + BN hybrid: ctiles 0-1 off-PE (GPSIMD scales 5
    taps into tmp tiles via tensor_scalar, DVE accumulates: tensor_scalar +
    3 scalar_tensor_tensor + 5 tensor_tensor adds, bf16), ctile 2 on PE as
    diagonal-stationary matmuls. BN bias + cast on ACT. This fills the
    DVE/GPSIMD idle window while PE runs the K/V phase, and shrinks PE's
    conv share.
  - K/V stride-2 convs on PE: 9 shifted diagonal-stationary matmuls
    accumulate in PSUM (diagonals built on ACT from identity x per-channel
    scale), BN bias folded in at the ACT evacuation.
  - K projection -> kh^T [o, T2] (ACT evac). V projection emitted
    TRANSPOSED (stationary = vf t-tile, moving = wv) producing vh^T [t, o]
    directly into vhT with a ones column per head (softmax denominator
    trick); no PE transposes.
  - Q projection on PE (softmax scale folded into wq) -> qh^T [o, T],
    chunked; chunks 2-6 are interleaved into attention band 0's head slots.
  - Attention per head: scores^T [t, q] = kh^T.T @ qh^T on PE, exp on ACT
    (no max-subtraction: |scores| << 1 by construction), o^T [65, q] =
    [vh | ones]^T @ e^T accumulated over t tiles. Denominator (psum row
    64) -> reciprocal_approx_fast on DVE -> partition_broadcast on GPSIMD
    (no DRAM bounce, no DMA) -> per-head evac multiply on DVE.
  - Output projection in [l, o] orientation; evacuation adds b_last
    (replicated tile) on DVE and DMAs straight to DRAM rows. The previous
    band's tiles are interleaved into the next band's head slots.
"""
import sys

sys.path.insert(0, '/opt/trn_rl_repo')

import numpy as np

DIM = 384
HEADS = 6
D = 64
S = 56           # stride-1 spatial side
S2 = 28          # stride-2 spatial side
T = S * S        # 3136
T2 = S2 * S2     # 784
EPS = 1e-5
SCALE = DIM ** -0.5
NCORES = 8
CT = DIM // 128          # 3 channel tiles
NTT = (T2 + 127) // 128  # 7 kv t-tiles (last = 16 rows)
QB = 1024                # attention q band width
# the narrow tail band runs second so its serial denominator chain overlaps
# a dense band instead of dangling at the kernel tail
BANDS = [(0, 1024), (3072, 64), (1024, 1024), (2048, 1024)]
QCHUNKS = [(0, 512), (512, 512), (1024, 512), (1536, 512), (2048, 512),
           (2560, 512), (3072, 64)]

TAPS = [(dy, dx) for dy in (-1, 0, 1) for dx in (-1, 0, 1)]  # k=(dy+1)*3+(dx+1)


def build_program():
    import concourse.mybir as mybir
    from concourse import bacc
    from concourse.tile import TileContext
    dt = mybir.dt
    AF = mybir.ActivationFunctionType
    ALU = mybir.AluOpType

    nc = bacc.Bacc()

    SP = S + 2
    xT = nc.dram_tensor("xT", [DIM, SP * SP], dt.bfloat16,
                        kind="ExternalInput")
    qcp = nc.dram_tensor("qcp", [DIM, 10], dt.float32, kind="ExternalInput")
    wqt = nc.dram_tensor("wqt", [DIM, DIM], dt.bfloat16, kind="ExternalInput")
    wk2 = nc.dram_tensor("wk2", [DIM, HEADS, 128], dt.bfloat16,
                         kind="ExternalInput")
    wv = nc.dram_tensor("wv", [DIM, DIM], dt.bfloat16, kind="ExternalInput")
    kvb = nc.dram_tensor("kvb", [DIM, 2], dt.float32, kind="ExternalInput")
    wlt = nc.dram_tensor("wlt", [DIM, DIM], dt.bfloat16, kind="ExternalInput")
    blast = nc.dram_tensor("blast", [1, DIM], dt.float32, kind="ExternalInput")
    diag = nc.dram_tensor("diag", [128, 9 * CT * 3, 128], dt.bfloat16,
                          kind="ExternalInput")
    out = nc.dram_tensor("out", [T, DIM], dt.float32, kind="ExternalOutput")

    with TileContext(nc) as tc:
        with (
            tc.tile_pool(name="const", bufs=1) as cpool,
            tc.tile_pool(name="ework", bufs=3) as epool,
        ):
            # ---------------- Phase 0: loads ----------------
            xT_sb = cpool.tile([128, CT, SP, SP], dt.bfloat16)
            qcp_sb = cpool.tile([128, CT, 10], dt.float32)
            kvb_sb = cpool.tile([128, CT, 2], dt.float32)
            wqt_sb = cpool.tile([128, CT, DIM], dt.bfloat16)
            wk2_sb = cpool.tile([128, CT, HEADS, 128], dt.bfloat16)
            wv_sb = cpool.tile([128, CT, DIM], dt.bfloat16)
            wlt_sb = cpool.tile([128, CT, DIM], dt.bfloat16)
            btile = cpool.tile([128, DIM], dt.float32)
            dk_sb = cpool.tile([128, 9 * CT, 128], dt.bfloat16)
            dv_sb = cpool.tile([128, 9 * CT, 128], dt.bfloat16)
            dq_sb = cpool.tile([128, 9 * CT, 128], dt.bfloat16)
            kf_sb = cpool.tile([128, CT, T2], dt.bfloat16)
            vf_sb = cpool.tile([128, CT, T2], dt.bfloat16)

            def csl(c):
                return slice(c * 128, (c + 1) * 128)

            # two parallel HWDGE streams: SP loads the K-conv path (needed
            # first), ACT loads the rest.
            nc.sync.dma_start(dk_sb[:], diag[:, 0:9 * CT, :])
            for c in range(CT):
                nc.sync.dma_start(
                    xT_sb[:, c, :, :],
                    xT[csl(c), :].rearrange("p (h w) -> p h w", w=SP))
            nc.sync.dma_start(dq_sb[:], diag[:, 9 * CT:18 * CT, :])
            nc.sync.dma_start(dv_sb[:], diag[:, 18 * CT:, :])
            for c in range(CT):
                nc.scalar.dma_start(qcp_sb[:, c, :], qcp[csl(c), :])
                nc.scalar.dma_start(kvb_sb[:, c, :], kvb[csl(c), :])
                nc.scalar.dma_start(wk2_sb[:, c, :, :], wk2[csl(c), :, :])
                nc.scalar.dma_start(wv_sb[:, c, :], wv[csl(c), :])
                nc.scalar.dma_start(wqt_sb[:, c, :], wqt[csl(c), :])
                nc.scalar.dma_start(wlt_sb[:, c, :], wlt[csl(c), :])
            nc.scalar.dma_start(btile[:],
                                blast[0:1, :].to_broadcast([128, DIM]))

            # persistent activations
            q_feat = cpool.tile([128, CT, T], dt.bfloat16)
            qh_sb = cpool.tile([128, CT, T], dt.bfloat16)
            kh2_sb = cpool.tile([128, HEADS, NTT * 128], dt.bfloat16)
            vhT_sb = cpool.tile([128, NTT, HEADS * 65], dt.bfloat16)
            o_sb = cpool.tile([128, CT, T], dt.bfloat16)

            v4 = vhT_sb[:].rearrange("p n (h c) -> p n h c", c=65)
            nc.gpsimd.memset(vhT_sb[:], 1.0)
            # zero the pad rows of the 16-row tail t-tile so AV can contract
            # over the full 128 partitions there (half-width stationaries run
            # the PE at half clock); stale e rows hit these zeros.
            nc.gpsimd.memset(vhT_sb[:, NTT - 1, :], 0.0)
            nc.vector.memset(v4[0:T2 - 6 * 128, NTT - 1, 0:HEADS, 64:65], 1.0)
            # zero the kh2 tail pad so the last scores tile is full-width:
            # its extra rows become exp(0)=1, nullified by the zeroed vhT pad.
            nc.gpsimd.memset(kh2_sb[:, :, T2:], 0.0)

            psA_cm = tc.tile_pool(name="psA", bufs=2, space="PSUM")
            psA = psA_cm.__enter__()
            psB_cm = tc.tile_pool(name="psB", bufs=2, space="PSUM")
            psB = psB_cm.__enter__()

            # ------------- Phase 2: K/V stride-2 conv + projections ---------
            def kv_conv(d_sb, f_sb, bias_col):
                for c in range(CT):
                    x5 = xT_sb[:, c, :, :].rearrange(
                        "p (h sy) (w sx) -> p h sy w sx", sy=2, sx=2)
                    ps = psA.tile([128, QB], dt.float32, tag="psA")
                    for half, (ha, hb) in enumerate(((0, 14), (14, 28))):
                        for k in range(9):
                            dy, dx = TAPS[k]
                            hoff, sy = ((0, 0) if dy == -1 else
                                        (0, 1) if dy == 0 else (1, 0))
                            woff, sx = ((0, 0) if dx == -1 else
                                        (0, 1) if dx == 0 else (1, 0))
                            nc.tensor.matmul(
                                ps[:, half * 512:half * 512 + 14 * S2],
                                d_sb[:, k * CT + c, :],
                                x5[:, ha + hoff:hb + hoff, sy,
                                   woff:woff + S2, sx],
                                start=(k == 0), stop=(k == 8))
                    nc.scalar.activation(
                        f_sb[:, c, :],
                        ps[:].rearrange("p (two u) -> p two u", two=2)
                        [:, :, 0:14 * S2],
                        AF.Identity,
                        bias=kvb_sb[:, c, bias_col:bias_col + 1])

            kv_conv(dk_sb, kf_sb, 0)
            # K projection -> kh2 [128, head, t]: head h's 64 dims land on
            # its parity rows, the other 64 rows are ZERO (host-zeroed wk2
            # columns) so scores can contract over the full 128 partitions
            # (the PE runs at half clock with 64-row stationaries).
            for h in range(HEADS):
                ps = psA.tile([128, QB], dt.float32, tag="psA")
                for half, (ha, hb) in enumerate(((0, 14), (14, 28))):
                    for c in range(CT):
                        nc.tensor.matmul(
                            ps[:, half * 512:half * 512 + 14 * S2],
                            wk2_sb[:, c, h, :],
                            kf_sb[:, c, ha * S2:hb * S2],
                            start=(c == 0), stop=(c == CT - 1))
                nc.scalar.activation(
                    kh2_sb[:, h, 0:T2]
                    .rearrange("p (two u) -> p two u", two=2),
                    ps[:].rearrange("p (two u) -> p two u", two=2)
                    [:, :, 0:14 * S2],
                    AF.Copy)

            # ---- Phase 1: Q conv on PE (diagonal stationaries, paired) ----
            QROWS = 8  # 8*56 = 448 free
            r0s = list(range(0, S, QROWS))
            for c in range(CT):
                x3 = xT_sb[:, c, :, :]
                for g in range(0, len(r0s), 2):
                    pair = r0s[g:g + 2]
                    ps = psA.tile([128, QB], dt.float32, tag="psA")
                    for half, r0 in enumerate(pair):
                        rows = min(QROWS, S - r0)
                        for k in range(9):
                            dy, dx = TAPS[k]
                            nc.tensor.matmul(
                                ps[:, half * 512:half * 512 + rows * S],
                                dq_sb[:, k * CT + c, :],
                                x3[:, 1 + dy + r0:1 + dy + r0 + rows,
                                   1 + dx:1 + dx + S],
                                start=(k == 0), stop=(k == 8))
                    if len(pair) == 2:
                        nc.scalar.activation(
                            q_feat[:, c, pair[0] * S:pair[0] * S
                                   + 2 * QROWS * S]
                            .rearrange("p (two u) -> p two u", two=2),
                            ps[:].rearrange("p (two u) -> p two u", two=2)
                            [:, :, 0:QROWS * S],
                            AF.Identity, bias=qcp_sb[:, c, 9:10])
                    else:
                        nc.scalar.activation(
                            q_feat[:, c, pair[0] * S:T],
                            ps[:, 0:(S - pair[0]) * S], AF.Identity,
                            bias=qcp_sb[:, c, 9:10])

            kv_conv(dv_sb, vf_sb, 1)
            # V projection TRANSPOSED: vh^T [t, o] = vf-tile^T @ wv, written
            # straight into the vhT layout (65-wide per head, ones preserved).
            for tt in range(NTT):
                tsz = min(128, T2 - tt * 128)
                ps = psB.tile([128, QB], dt.float32, tag="psB")
                for c in range(CT):
                    nc.tensor.matmul(
                        ps[0:tsz, 0:DIM],
                        vf_sb[:, c, tt * 128:tt * 128 + tsz],
                        wv_sb[:, c, :],
                        start=(c == 0), stop=(c == CT - 1))
                nc.scalar.activation(
                    v4[0:tsz, tt, 0:HEADS, 0:64],
                    ps[0:tsz, 0:DIM].rearrange("p (h c) -> p h c", c=64),
                    AF.Copy)

            # ---------------- Phase 3: Q projection (qh^T [o, T]) -----------
            def qproj_chunk(lc):
                lpos, lw = QCHUNKS[lc]
                for ot in range(CT):
                    osl = slice(ot * 128, (ot + 1) * 128)
                    ps = psA.tile([128, QB], dt.float32, tag="psA")
                    for c in range(CT):
                        nc.tensor.matmul(
                            ps[:, 0:lw], wqt_sb[:, c, osl],
                            q_feat[:, c, lpos:lpos + lw],
                            start=(c == 0), stop=(c == CT - 1))
                    nc.vector.tensor_copy(qh_sb[:, ot, lpos:lpos + lw],
                                          ps[:, 0:lw])

            # chunks 0-1 paired into one PSUM tile (one DVE evac for 1024)
            for ot in range(CT):
                osl = slice(ot * 128, (ot + 1) * 128)
                ps = psA.tile([128, QB], dt.float32, tag="psA")
                for half in (0, 1):
                    lpos, lw = QCHUNKS[half]
                    for c in range(CT):
                        nc.tensor.matmul(
                            ps[:, half * 512:half * 512 + lw],
                            wqt_sb[:, c, osl],
                            q_feat[:, c, lpos:lpos + lw],
                            start=(c == 0), stop=(c == CT - 1))
                nc.vector.tensor_copy(qh_sb[:, ot, 0:1024], ps[:])

            # ---------------- Phase 4: attention ----------------
            def oproj_tile(lpos, lsz):
                ps = psB.tile([128, QB], dt.float32, tag="psB")
                for c in range(CT):
                    nc.tensor.matmul(
                        ps[0:lsz, 0:DIM], o_sb[:, c, lpos:lpos + lsz],
                        wlt_sb[:, c, :],
                        start=(c == 0), stop=(c == CT - 1))
                ostage = epool.tile([128, DIM], dt.float32, tag="ostage",
                                    bufs=2)
                nc.vector.tensor_tensor(
                    out=ostage[0:lsz, :], in0=ps[0:lsz, 0:DIM],
                    in1=btile[0:lsz, :], op=ALU.add)
                nc.sync.dma_start(out[lpos:lpos + lsz, :], ostage[0:lsz, :])

            def band_ltiles(qs, W):
                return [(qs + i, min(128, qs + W - (qs + i)))
                        for i in range(0, W, 128)]

            def scores_mk(h, qs, W):
                ot = h // 2

                def scores(tt):
                    ps_s = psA.tile([128, QB], dt.float32, tag="psA")
                    for sub in range(0, W, 512):
                        sw = min(512, W - sub)
                        nc.tensor.matmul(
                            ps_s[:, sub:sub + sw],
                            kh2_sb[:, h, tt * 128:tt * 128 + 128],
                            qh_sb[:, ot, qs + sub:qs + sub + sw],
                            start=True, stop=True)
                    return ps_s

                return scores

            def head_tloop(h, qs, W, ps_o, obase, ps_s0=None, prefetch=None):
                """scores -> exp -> o accumulation for one head over all
                t-tiles, software-pipelined so PE never stalls on ACT.
                ps_s0: pre-computed scores(0) from the previous head's loop;
                prefetch(): emits the NEXT head's scores(0) inside this loop
                (cross-head software pipelining), returned to the caller."""
                scores = scores_mk(h, qs, W)
                next_ps = None
                ps_s = ps_s0 if ps_s0 is not None else scores(0)
                for tt in range(NTT):
                    e = epool.tile([128, QB], dt.bfloat16, tag="e")
                    nc.scalar.activation(e[:, 0:W], ps_s[:, 0:W], AF.Exp)
                    if tt + 1 < NTT:
                        ps_s = scores(tt + 1)
                    elif prefetch is not None:
                        next_ps = prefetch()
                    for sub in range(0, W, 512):
                        sw = min(512, W - sub)
                        nc.tensor.matmul(
                            ps_o[0:65, obase + sub:obase + sub + sw],
                            vhT_sb[0:128, tt, h * 65:h * 65 + 65],
                            e[0:128, sub:sub + sw],
                            start=(tt == 0), stop=(tt == NTT - 1))
                return next_ps

            def norm_chain(ps_o, WW):
                """den row 64 -> SBUF -> reciprocal_approx_fast (DVE; its
                bitwise seed misreads PSUM directly) -> broadcast to 64
                partitions (GPSIMD). No DMA, no DRAM bounce."""
                den_sb = epool.tile([1, QB], dt.float32, tag="den", bufs=2)
                r_row = epool.tile([1, QB], dt.float32, tag="r_row", bufs=2)
                r_rep = epool.tile([64, QB], dt.float32, tag="r_rep", bufs=2)
                nc.vector.tensor_copy(den_sb[0:1, 0:WW], ps_o[64:65, 0:WW])
                nc.vector.reciprocal_approx_fast(r_row[0:1, 0:WW],
                                                 den_sb[0:1, 0:WW])
                nc.gpsimd.partition_broadcast(r_rep[0:64, 0:WW],
                                              r_row[0:1, 0:WW])
                return r_rep

            def evac_head(h, qs, W, ps_o, obase, r_rep, rbase):
                ot = h // 2
                hsl = slice(64 * (h % 2), 64 * (h % 2) + 64)
                nc.vector.tensor_tensor(
                    out=o_sb[hsl, ot, qs:qs + W],
                    in0=ps_o[0:64, obase:obase + W],
                    in1=r_rep[0:64, rbase:rbase + W],
                    op=ALU.mult)

            # filler work interleaved into head slots: band 0 gets the
            # remaining Q projection chunks; later bands get the previous
            # band's output-projection tiles.
            prev_band = None
            first_band = True
            for qs, W in BANDS:
                if first_band:
                    fillers = [(lambda lc=lc: qproj_chunk(lc))
                               for lc in range(2, len(QCHUNKS))]
                else:
                    fillers = [(lambda lp=lp, ls=ls: oproj_tile(lp, ls))
                               for lp, ls in band_ltiles(*prev_band)]

                if W * HEADS <= 512:
                    # narrow tail band: all heads share one PSUM tile and a
                    # single denominator chain.
                    ps_o = psB.tile([128, QB], dt.float32, tag="psB")
                    for h in range(HEADS):
                        head_tloop(h, qs, W, ps_o, h * W)
                        if h < len(fillers):
                            fillers[h]()
                    r_rep = norm_chain(ps_o, W * HEADS)
                    for h in range(HEADS):
                        evac_head(h, qs, W, ps_o, h * W, r_rep, h * W)
                else:
                    # cross-head prefetch only when fillers don't touch psA
                    # (band 0's qproj fillers would collide with the
                    # prefetched scores tile in the 2-deep psA ring).
                    can_pf = not first_band
                    next_ps = None
                    for h in range(HEADS):
                        ps_o = psB.tile([128, QB], dt.float32, tag="psB")
                        pf = (scores_mk(h + 1, qs, W)
                              if can_pf and h + 1 < HEADS else None)
                        next_ps = head_tloop(
                            h, qs, W, ps_o, 0, ps_s0=next_ps,
                            prefetch=(lambda f=pf: f(0)) if pf else None)
                        r_rep = norm_chain(ps_o, W)
                        evac_head(h, qs, W, ps_o, 0, r_rep, 0)
                        if h < len(fillers):
                            fillers[h]()

                for f in fillers[HEADS:]:
                    f()
                prev_band = (qs, W)
                first_band = False

            for lt in band_ltiles(*prev_band):
                oproj_tile(*lt)

            psB_cm.__exit__(None, None, None)
            psA_cm.__exit__(None, None, None)

    nc.compile()
    return nc


_CACHE = {}


def _prep_weights(inputs):
    import ml_dtypes
    bf16 = ml_dtypes.bfloat16
    f32 = np.float32

    def bn_fold(prefix):
        a = (np.asarray(inputs[f'bn{prefix}_s'], f32)
             / np.sqrt(np.asarray(inputs[f'bn{prefix}_v'], f32) + EPS))
        b = (np.asarray(inputs[f'bn{prefix}_b'], f32)
             - np.asarray(inputs[f'bn{prefix}_m'], f32) * a)
        return a.astype(f32), b.astype(f32)

    aq, bq = bn_fold('q')
    ak, bk = bn_fold('k')
    av, bv = bn_fold('v')

    conv_q = np.asarray(inputs['conv_q'], f32)[:, 0].reshape(DIM, 9)
    conv_k = np.asarray(inputs['conv_k'], f32)[:, 0].reshape(DIM, 9)
    conv_v = np.asarray(inputs['conv_v'], f32)[:, 0].reshape(DIM, 9)
    wq = np.asarray(inputs['wq'], f32)
    wk = np.asarray(inputs['wk'], f32)
    wv = np.asarray(inputs['wv'], f32)
    wl = np.asarray(inputs['w_last'], f32)

    qcp = np.zeros((DIM, 10), f32)
    qcp[:, :9] = conv_q * aq[:, None]
    qcp[:, 9] = bq

    qs_ = conv_q * aq[:, None]
    ks_ = conv_k * ak[:, None]
    vs_ = conv_v * av[:, None]
    diag = np.zeros((128, 81, 128), f32)
    ar = np.arange(128)
    for c in range(CT):
        for k in range(9):
            diag[ar, k * CT + c, ar] = ks_[128 * c + ar, k]
            diag[ar, 9 * CT + k * CT + c, ar] = qs_[128 * c + ar, k]
            diag[ar, 18 * CT + k * CT + c, ar] = vs_[128 * c + ar, k]

    wqt = np.ascontiguousarray((wq * SCALE).T).astype(bf16)  # [c, o]
    wkT = wk.T.astype(np.float32)  # [c, o]
    wk2 = np.zeros((DIM, HEADS, 128), np.float32)
    for h in range(HEADS):
        p = h % 2
        wk2[:, h, 64 * p:64 * p + 64] = wkT[:, 64 * h:64 * h + 64]
    wk2 = wk2.astype(bf16)
    wvt = np.ascontiguousarray(wv.T).astype(bf16)
    kvb = np.stack([bk, bv], axis=1).astype(f32)        # [c, 2]
    wlt = np.ascontiguousarray(wl.T).astype(bf16)
    blast = np.asarray(inputs['b_last'], f32).reshape(1, DIM)
    return {'qcp': qcp, 'wqt': wqt, 'wk2': wk2, 'wv': wvt,
            'kvb': kvb, 'wlt': wlt, 'blast': blast,
            'diag': diag.astype(bf16)}


def _prep_x(xb):
    """[T, C] f32 -> zero-padded transposed [C, 58*58] bf16."""
    import ml_dtypes
    pad = np.zeros((DIM, S + 2, S + 2), np.float32)
    pad[:, 1:1 + S, 1:1 + S] = xb.T.reshape(DIM, S, S)
    return pad.reshape(DIM, (S + 2) * (S + 2)).astype(ml_dtypes.bfloat16)


def kernel(**inputs):
    from concourse.bass_utils import run_bass_kernel_spmd

    if 'nc' not in _CACHE:
        _CACHE['nc'] = build_program()
    nc = _CACHE['nc']

    wmap = _prep_weights(inputs)
    x = np.asarray(inputs['x'], np.float32)  # [8, T, C]
    B = x.shape[0]

    in_maps = [{'xT': _prep_x(x[b]), **wmap} for b in range(B)]

    res = run_bass_kernel_spmd(nc, in_maps, list(range(NCORES)))
    outs = np.stack([np.asarray(res.results[b]['out']) for b in range(B)],
                    axis=0)
    return outs.astype(np.float32)
